# revision 47
# baseline (speedup 1.0000x reference)
"""DenseCorr2d full kernel for 8 Trainium2 NeuronCores.

Reference computation (per example b):
  corr[(cm*16+ct), y, x] = sum_{dy,dx} tm_edgepad[cm, y+dy, x+dx] * tp[ct, dy, dx]
  out[co, y, x] = bias[co] + sum_{ci,ky,kx} W[co, ci, ky, kx] * corr_zpad[ci, y+ky-1, x+kx-1]

Sharding: data-parallel over batch; core i computes example i entirely.

Stage A (dense correlation) folds the template taps into the matmul
contraction dim: with y = 8w + j and dx = 4a + de, the contraction rows are
(f = j+dy, de) = 92 partitions, the stationary columns are (ct, j) = 128
(fully dense), and accumulation over a happens in PSUM (4 matmuls per
128-col x 512-spatial tile).  The moving operand baseT[cm, (f,de), w, x'] =
tm_pad[cm, 8w+f, x'+de] is precomputed on host so each tile load is a
contiguous-per-partition DMA.  bt loads ride the two HWDGE queues (sync:
cm2=0, scalar: cm2=1) as maximal-size DMAs, all issued up front;
PSUM is allocated per-cm so evacuation of cm N overlaps the matmuls of
cm N+1.

corr lives in SBUF j-interleaved: corr[ci, (c, j, ws, x)] holds row
y = 8*(ws-1) + j of chunk c (ws=0/17 are constant zero rows, memset once
up front; x is 130 wide with zero x-borders).  The PSUM evacuation is a
full-rate copy (cast fp32->bf16 into a staging tile) plus two half-slab
DMAs per cm on the gpsimd SWDGE rings (DMA engines 4-15), keeping the 4
engines behind the HWDGE queues free for bt loads.  The kernel is DMA-
bound in this phase, so the chunk-0 halves of ALL stage-B blocks are
interleaved into pairs 4-7 (3 per cm): they fill the PE's DMA-stall
windows, their bf16 partials parking in SBUF.  Pass 2 then runs only the
chunk-1 taps and folds partial+bias into the psum evacuation with one
DVE scalar_tensor_tensor per block.

Stage B runs the 3x3 'same' merge conv over residue bands as ROW-PAIRED
matmuls: one [128, 2, 64] stationary holds the weights for two adjacent
output rows (j0, j0+1) that share the same corr slab row jj = j0+dd-1
(slot s covers row j0+s with ky = dd-s; invalid (dd,s) combos hold zero
weights).  That cuts the matmuls per block from 18 to 12 per output row
and uses the full 128 PE columns on the interior taps.  Each slot
evacuates to its own partition range of the output tile (scalar adds the
bias), and the output DMA un-interleaves the (slot, co) partitions into
the right y rows.

All matmuls run in bf16 (inputs are unit-normal; accumulation in fp32
PSUM keeps the relative error ~4e-3, well inside the 2e-2 gate).
"""

from contextlib import ExitStack

import ml_dtypes
import numpy as np

import concourse.bass as bass
import concourse.tile as tile
from concourse import bacc, mybir
from concourse.bass_utils import run_bass_kernel_spmd

F32 = mybir.dt.float32
BF16 = mybir.dt.bfloat16

N_CORES = 8
# Problem shapes (hardcoded per contract).
B, CT, HT, WT = 8, 16, 16, 16
CM, HM, WM = 16, 128, 128
COUT, K = 64, 3
HP = HM + HT - 1  # 143 padded image rows/cols
NF = 23  # f = j + dy range
NP = 4 * NF  # 92 contraction rows (f, de); de in [0,4), dx = 4a + de
NA = 4  # PSUM accumulation steps over a
XW = 140  # x' range of baseT (x + 4a <= 127+12)
SLAB = 18 * 130  # corr slab (c, j): 18 ws-rows of 130
NCORR = 16 * SLAB  # 2 chunks * 8 j

# stage-B w-window split: 16 w values per j0-residue, psum <= 512 fp32
RWS = [3, 3, 3, 3, 2, 2]
W0S = [0, 3, 6, 9, 12, 14]

_CACHE: dict = {}


def _emit(ctx: ExitStack, tc, nc, btT, sa2, wst, bia, zzb, out):
    const = ctx.enter_context(tc.tile_pool(name="const", bufs=1))
    corrp = ctx.enter_context(tc.tile_pool(name="corrp", bufs=1))

    # sa2 rides the scalar queue so the sync queue's first op is the first
    # bt load; both are needed by matmul #1.  w goes to the otherwise-idle
    # gpsimd SWDGE so it never delays loads or shuffles.
    sa2_sb = const.tile([NP, NA, 128], BF16, name="sa2_sb")
    nc.scalar.dma_start(out=sa2_sb[:], in_=sa2.ap())
    b_sb = const.tile([128, 1], F32, name="b_sb")
    nc.scalar.dma_start(out=b_sb[:], in_=bia.ap())
    # row-paired stage-B weights: [k, (c, dd, kx), s, co]
    w_sb = const.tile([128, 24, 2, COUT], BF16, name="w_sb")
    nc.gpsimd.dma_start(out=w_sb[:, :12], in_=wst.ap()[:, :12])
    nc.gpsimd.dma_start(out=w_sb[:, 12:], in_=wst.ap()[:, 12:])

    corr_sb = corrp.tile([128, NCORR + 2], BF16, name="corr_sb")
    corr_flat = corr_sb[:]
    # slab view: [p, c*8+j, ws*130+x]
    corr_j = corr_sb[:, :NCORR].rearrange("p (s t) -> p s t", s=16)
    corr_r = corr_sb[:, :NCORR].rearrange("p (s w x) -> p s w x", s=16, w=18)
    # the ws=0/17 zero-pad rows are constant: memset once up front instead of
    # shuffling 2 zero rows per cm (-11% shuffle bytes)
    nc.gpsimd.memset(corr_r[:, :, 0, :], 0.0)
    nc.gpsimd.memset(corr_r[:, :, 17, :], 0.0)
    # tail slack
    nc.scalar.dma_start(out=corr_sb[:, NCORR:], in_=zzb.ap()[:, :2])

    # ---- Stage B structure (needed during stage A for the interleave) ----
    def slab_off(c, jj, w0):
        if jj < 0:
            return (c * 8 + 7) * SLAB + w0 * 130
        if jj > 7:
            return (c * 8 + jj - 8) * SLAB + (w0 + 2) * 130
        return (c * 8 + jj) * SLAB + (w0 + 1) * 130

    # out dram view for the un-interleave: [s, co, j0p, w, x]
    out_v = out.ap().rearrange("co (w jp s) x -> s co jp w x", jp=4, s=2)
    blocks = [(wg, j0p) for wg in range(len(RWS)) for j0p in range(4)]
    NPART = 24  # blocks whose chunk-0 half runs inside stage A
    psB = ctx.enter_context(tc.tile_pool(name="psB", bufs=2, space="PSUM"))
    outp = ctx.enter_context(tc.tile_pool(name="outp", bufs=4))
    bpp = ctx.enter_context(tc.tile_pool(name="bpp", bufs=1))
    # bf16 partials of the interleaved blocks (all rw=3 -> n=390); the
    # half-precision parking adds ~0.3% RMS, far inside the 2e-2 gate
    bpx = bpp.tile([128, NPART, 390], BF16, name="bpx")

    def emit_c0_partial(k):
        # chunk-0 half of block k, run inside stage A's DMA-stall windows;
        # the bf16 partial parks in SBUF until pass 2.
        wg, j0p = blocks[k]
        n = RWS[wg] * 130
        pb = psB.tile([128, n], F32, name="pb", tag="pb")
        for dd in range(4):
            off0 = slab_off(0, 2 * j0p + dd - 1, W0S[wg])
            for kx in range(3):
                nc.tensor.matmul(
                    pb[:],
                    w_sb[:, dd * 3 + kx, :, :],
                    corr_flat[:, off0 + kx : off0 + kx + n],
                    start=(dd == 0 and kx == 0),
                    stop=(dd == 3 and kx == 2),
                )
        if k % 2 == 0:
            nc.scalar.copy(bpx[:, k, :n], pb[:])
        else:
            nc.vector.tensor_copy(bpx[:, k, :n], pb[:])

    # ---- Stage A ----
    with (
        tc.tile_pool(name="btp", bufs=7) as btp,
        tc.tile_pool(name="stp0", bufs=4) as stp0,
        tc.tile_pool(name="stp1", bufs=4) as stp1,
        tc.tile_pool(name="psA", bufs=6, space="PSUM") as psA,
    ):
        stps = [stp0, stp1]

        def load_pair(pr):
            # big per-cm2-half DMAs, cm2=0 on sync / cm2=1 on scalar: the
            # two HWDGE queues pull in parallel (each sustains ~30GB/s;
            # one queue alone cannot feed the PE).
            t = btp.tile([NP, 2, 16, XW], BF16, name="bt", tag="bt")
            if pr == 0:
                nc.sync.dma_start(out=t[:, 0, 0:8], in_=btT.ap()[0, :, 0, 0:8])
                nc.sync.dma_start(out=t[:, 0, 8:16], in_=btT.ap()[0, :, 0, 8:16])
            else:
                nc.sync.dma_start(out=t[:, 0], in_=btT.ap()[pr, :, 0])
            nc.scalar.dma_start(out=t[:, 1], in_=btT.ap()[pr, :, 1])
            return t

        loads = {pr: load_pair(pr) for pr in range(8)}
        for pr in range(8):
            bt = loads.pop(pr)
            for cm2 in range(2):
                cm = 2 * pr + cm2
                pts = [
                    psA.tile([128, 4, 128], F32, name=f"pA{ybg}", tag="pA")
                    for ybg in range(4)
                ]
                # a-inner so each ybg-block only needs its own bt chunk; the
                # 4 accumulating matmuls to one bank run back-to-back.
                for ybg in range(4):
                    for a in range(NA):
                        nc.tensor.matmul(
                            pts[ybg][:],
                            sa2_sb[:, a, :],
                            bt[:, cm2, 4 * ybg : 4 * ybg + 4, 4 * a : 4 * a + 128],
                            start=(a == 0),
                            stop=(a == NA - 1),
                        )
                # st carries only the 16 data rows (slab rows 1..16); the
                # constant ws=0/17 pad rows were memset once up front.
                st = stps[cm2].tile([128, 16, 130], BF16, name="st", tag="st")
                nc.vector.memset(st[:, :, 0], 0.0)
                nc.vector.memset(st[:, :, 129], 0.0)
                for ybg in range(4):
                    dst_sl = st[:, 4 * ybg : 4 * ybg + 4, 1:129]
                    if ybg % 2 == 0:
                        nc.vector.tensor_copy(dst_sl, pts[ybg][:])
                    else:
                        nc.scalar.copy(dst_sl, pts[ybg][:])
                # st[(8ct+j), ws, x] -> corr slabs (c, j), split by ct-half.
                # Shuffles ride gpsimd SWDGE rings (DMA engines 4-15, which
                # sit idle) so the 4 engines behind the HWDGE queues carry
                # only bt loads; the last cms use the by-then-idle HWDGE
                # queues so the SWDGE path is never the stage-A tail.
                c, cmh = cm // 8, cm % 8
                if cm >= 14:
                    # quarters across all queues so the tail shuffles land
                    # fast (loads are done; HWDGE engines are idle by now)
                    for qi, eng in enumerate(
                        (nc.sync, nc.scalar, nc.gpsimd, nc.gpsimd)
                    ):
                        eng.dma_start(
                            out=corr_j[
                                16 * cmh + 4 * qi : 16 * cmh + 4 * qi + 4,
                                c * 8 : c * 8 + 8,
                                130 : 130 + 16 * 130,
                            ],
                            in_=st[32 * qi : 32 * qi + 32],
                        )
                else:
                    # both halves on gpsimd SWDGE (DMA engines 4-15): the
                    # HWDGE queues' 4 engines carry only bt loads
                    e0, e1 = nc.gpsimd, nc.gpsimd
                    e0.dma_start(
                        out=corr_j[
                            16 * cmh : 16 * cmh + 8,
                            c * 8 : c * 8 + 8,
                            130 : 130 + 16 * 130,
                        ],
                        in_=st[0:64],
                    )
                    e1.dma_start(
                        out=corr_j[
                            16 * cmh + 8 : 16 * cmh + 16,
                            c * 8 : c * 8 + 8,
                            130 : 130 + 16 * 130,
                        ],
                        in_=st[64:128],
                    )
                if pr >= 4:
                    # fill this cm's trailing DMA-stall window with stage-B
                    # chunk-0 work (its corr half has long been shuffled)
                    bk = 3 * (2 * (pr - 4) + cm2)
                    for d in range(3):
                        emit_c0_partial(bk + d)

    # ---- Stage B pass 2: chunk-1 taps; partialed blocks add their parked
    # chunk-0 sums in the combine op.  Stage A's PSUM banks are free now, so
    # a 4-deep pool hides the combine/out-DMA latency. ----
    psB2 = ctx.enter_context(tc.tile_pool(name="psB2", bufs=4, space="PSUM"))
    for k, (wg, j0p) in enumerate(blocks):
        w0, rw = W0S[wg], RWS[wg]
        n = rw * 130
        pb = psB2.tile([128, n], F32, name="pb2", tag="pb2")
        cs = (1,) if k < NPART else (0, 1)
        for c in cs:
            for dd in range(4):
                off0 = slab_off(c, 2 * j0p + dd - 1, w0)
                for kx in range(3):
                    nc.tensor.matmul(
                        pb[:],
                        w_sb[:, c * 12 + dd * 3 + kx, :, :],
                        corr_flat[:, off0 + kx : off0 + kx + n],
                        start=(c == cs[0] and dd == 0 and kx == 0),
                        stop=(c == 1 and dd == 3 and kx == 2),
                    )
        ot = outp.tile([128, rw, WM], F32, name="ot", tag="ot")
        pbv = pb[:].rearrange("p (a b) -> p a b", b=130)[:, :, 0:128]
        if k < NPART:
            # out = (pb + bias) + parked chunk-0 partial, one DVE op
            nc.vector.scalar_tensor_tensor(
                ot[:],
                pbv,
                b_sb[:, 0:1],
                bpx[:, k, :n].rearrange("p (a b) -> p a b", b=130)[:, :, 0:128],
                mybir.AluOpType.add,
                mybir.AluOpType.add,
            )
        else:
            nc.scalar.activation(
                ot[:],
                pbv,
                mybir.ActivationFunctionType.Identity,
                bias=b_sb[:, 0:1],
            )
        for sl in range(2):
            eng = nc.sync if sl == 0 else nc.scalar
            eng.dma_start(
                out=out_v[sl, :, j0p, w0 : w0 + rw, :],
                in_=ot[64 * sl : 64 * sl + 64],
            )


# revision 48
# speedup vs baseline: 1.0329x; 1.0329x over previous
"""DenseCorr2d full kernel for 8 Trainium2 NeuronCores.

Reference computation (per example b):
  corr[(cm*16+ct), y, x] = sum_{dy,dx} tm_edgepad[cm, y+dy, x+dx] * tp[ct, dy, dx]
  out[co, y, x] = bias[co] + sum_{ci,ky,kx} W[co, ci, ky, kx] * corr_zpad[ci, y+ky-1, x+kx-1]

Sharding: data-parallel over batch; core i computes example i entirely.

Stage A (dense correlation) folds the template taps into the matmul
contraction dim: with y = 8w + j and dx = 4a + de, the contraction rows are
(f = j+dy, de) = 92 partitions, the stationary columns are (ct, j) = 128
(fully dense), and accumulation over a happens in PSUM (4 matmuls per
128-col x 512-spatial tile).  The moving operand baseT[cm, (f,de), w, x'] =
tm_pad[cm, 8w+f, x'+de] is precomputed on host so each tile load is a
contiguous-per-partition DMA.  bt loads ride the two HWDGE queues (sync:
cm2=0, scalar: cm2=1) as maximal-size DMAs, all issued up front;
PSUM is allocated per-cm so evacuation of cm N overlaps the matmuls of
cm N+1.

corr lives in SBUF j-interleaved: corr[ci, (c, j, ws, x)] holds row
y = 8*(ws-1) + j of chunk c (ws=0/17 are constant zero rows, memset once
up front; x is 130 wide with zero x-borders).  The PSUM evacuation is a
full-rate copy (cast fp32->bf16 into a staging tile) plus two half-slab
DMAs per cm on the gpsimd SWDGE rings (DMA engines 4-15), keeping the 4
engines behind the HWDGE queues free for bt loads.  The kernel is DMA-
bound in this phase, so the chunk-0 halves of ALL stage-B blocks are
interleaved into pairs 4-7 (3 per cm): they fill the PE's DMA-stall
windows, their bf16 partials parking in SBUF.  Pass 2 then runs only the
chunk-1 taps and folds partial+bias into the psum evacuation with one
DVE scalar_tensor_tensor per block.

Stage B runs the 3x3 'same' merge conv over residue bands as ROW-PAIRED
matmuls: one [128, 2, 64] stationary holds the weights for two adjacent
output rows (j0, j0+1) that share the same corr slab row jj = j0+dd-1
(slot s covers row j0+s with ky = dd-s; invalid (dd,s) combos hold zero
weights).  That cuts the matmuls per block from 18 to 12 per output row
and uses the full 128 PE columns on the interior taps.  Each slot
evacuates to its own partition range of the output tile (scalar adds the
bias), and the output DMA un-interleaves the (slot, co) partitions into
the right y rows.

All matmuls run in bf16 (inputs are unit-normal; accumulation in fp32
PSUM keeps the relative error ~4e-3, well inside the 2e-2 gate).
"""

from contextlib import ExitStack

import ml_dtypes
import numpy as np

import concourse.bass as bass
import concourse.tile as tile
from concourse import bacc, mybir
from concourse.bass_utils import run_bass_kernel_spmd

F32 = mybir.dt.float32
BF16 = mybir.dt.bfloat16

N_CORES = 8
# Problem shapes (hardcoded per contract).
B, CT, HT, WT = 8, 16, 16, 16
CM, HM, WM = 16, 128, 128
COUT, K = 64, 3
HP = HM + HT - 1  # 143 padded image rows/cols
NF = 23  # f = j + dy range
NP = 4 * NF  # 92 contraction rows (f, de); de in [0,4), dx = 4a + de
NA = 4  # PSUM accumulation steps over a
XW = 140  # x' range of baseT (x + 4a <= 127+12)
SLAB = 18 * 130  # corr slab (c, j): 18 ws-rows of 130
NCORR = 16 * SLAB  # 2 chunks * 8 j

# stage-B w-window split: 16 w values per j0-residue, psum <= 512 fp32
RWS = [3, 3, 3, 3, 2, 2]
W0S = [0, 3, 6, 9, 12, 14]

_CACHE: dict = {}


def _emit(ctx: ExitStack, tc, nc, btT, sa2, wst, bia, zzb, out):
    const = ctx.enter_context(tc.tile_pool(name="const", bufs=1))
    corrp = ctx.enter_context(tc.tile_pool(name="corrp", bufs=1))

    # sa2 rides the scalar queue so the sync queue's first op is the first
    # bt load; both are needed by matmul #1.  w goes to the otherwise-idle
    # gpsimd SWDGE so it never delays loads or shuffles.
    sa2_sb = const.tile([NP, NA, 128], BF16, name="sa2_sb")
    nc.scalar.dma_start(out=sa2_sb[:], in_=sa2.ap())
    b_sb = const.tile([128, 1], F32, name="b_sb")
    nc.scalar.dma_start(out=b_sb[:], in_=bia.ap())
    # row-paired stage-B weights: [k, (c, dd, kx), s, co]
    w_sb = const.tile([128, 24, 2, COUT], BF16, name="w_sb")
    nc.gpsimd.dma_start(out=w_sb[:, :12], in_=wst.ap()[:, :12])
    nc.gpsimd.dma_start(out=w_sb[:, 12:], in_=wst.ap()[:, 12:])

    corr_sb = corrp.tile([128, NCORR + 2], BF16, name="corr_sb")
    corr_flat = corr_sb[:]
    # slab view: [p, c*8+j, ws*130+x]
    corr_j = corr_sb[:, :NCORR].rearrange("p (s t) -> p s t", s=16)
    corr_r = corr_sb[:, :NCORR].rearrange("p (s w x) -> p s w x", s=16, w=18)
    # the ws=0/17 zero-pad rows are constant: memset once up front instead of
    # shuffling 2 zero rows per cm (-11% shuffle bytes)
    nc.gpsimd.memset(corr_r[:, :, 0, :], 0.0)
    nc.gpsimd.memset(corr_r[:, :, 17, :], 0.0)
    # tail slack
    nc.scalar.dma_start(out=corr_sb[:, NCORR:], in_=zzb.ap()[:, :2])

    # ---- Stage B structure (needed during stage A for the interleave) ----
    def slab_off(c, jj, w0):
        if jj < 0:
            return (c * 8 + 7) * SLAB + w0 * 130
        if jj > 7:
            return (c * 8 + jj - 8) * SLAB + (w0 + 2) * 130
        return (c * 8 + jj) * SLAB + (w0 + 1) * 130

    # out dram view for the un-interleave: [s, co, j0p, w, x]
    out_v = out.ap().rearrange("co (w jp s) x -> s co jp w x", jp=4, s=2)
    blocks = [(wg, j0p) for wg in range(len(RWS)) for j0p in range(4)]
    NPART = 24  # blocks whose chunk-0 half runs inside stage A
    psB = ctx.enter_context(tc.tile_pool(name="psB", bufs=2, space="PSUM"))
    outp = ctx.enter_context(tc.tile_pool(name="outp", bufs=4))
    bpp = ctx.enter_context(tc.tile_pool(name="bpp", bufs=1))
    # bf16 partials of the interleaved blocks (all rw=3 -> n=390); the
    # half-precision parking adds ~0.3% RMS, far inside the 2e-2 gate
    bpx = bpp.tile([128, NPART, 390], BF16, name="bpx")

    def emit_c0_partial(k):
        # chunk-0 half of block k, run inside stage A's DMA-stall windows;
        # the bf16 partial parks in SBUF until pass 2.
        wg, j0p = blocks[k]
        n = RWS[wg] * 130
        pb = psB.tile([128, n], F32, name="pb", tag="pb")
        for dd in range(4):
            off0 = slab_off(0, 2 * j0p + dd - 1, W0S[wg])
            for kx in range(3):
                nc.tensor.matmul(
                    pb[:],
                    w_sb[:, dd * 3 + kx, :, :],
                    corr_flat[:, off0 + kx : off0 + kx + n],
                    start=(dd == 0 and kx == 0),
                    stop=(dd == 3 and kx == 2),
                )
        if k % 2 == 0:
            nc.scalar.copy(bpx[:, k, :n], pb[:])
        else:
            nc.vector.tensor_copy(bpx[:, k, :n], pb[:])

    # ---- Stage A ----
    with (
        tc.tile_pool(name="btp", bufs=8) as btp,
        tc.tile_pool(name="stp0", bufs=3) as stp0,
        tc.tile_pool(name="stp1", bufs=3) as stp1,
        tc.tile_pool(name="psA", bufs=6, space="PSUM") as psA,
    ):
        stps = [stp0, stp1]

        def load_pair(pr):
            # big per-cm2-half DMAs, cm2=0 on sync / cm2=1 on scalar: the
            # two HWDGE queues pull in parallel (each sustains ~30GB/s;
            # one queue alone cannot feed the PE).
            t = btp.tile([NP, 2, 16, XW], BF16, name="bt", tag="bt")
            if pr == 0:
                nc.sync.dma_start(out=t[:, 0, 0:8], in_=btT.ap()[0, :, 0, 0:8])
                nc.sync.dma_start(out=t[:, 0, 8:16], in_=btT.ap()[0, :, 0, 8:16])
            else:
                nc.sync.dma_start(out=t[:, 0], in_=btT.ap()[pr, :, 0])
            nc.scalar.dma_start(out=t[:, 1], in_=btT.ap()[pr, :, 1])
            return t

        loads = {pr: load_pair(pr) for pr in range(8)}
        for pr in range(8):
            bt = loads.pop(pr)
            for cm2 in range(2):
                cm = 2 * pr + cm2
                pts = [
                    psA.tile([128, 4, 128], F32, name=f"pA{ybg}", tag="pA")
                    for ybg in range(4)
                ]
                # a-inner so each ybg-block only needs its own bt chunk; the
                # 4 accumulating matmuls to one bank run back-to-back.
                for ybg in range(4):
                    for a in range(NA):
                        nc.tensor.matmul(
                            pts[ybg][:],
                            sa2_sb[:, a, :],
                            bt[:, cm2, 4 * ybg : 4 * ybg + 4, 4 * a : 4 * a + 128],
                            start=(a == 0),
                            stop=(a == NA - 1),
                        )
                # st carries only the 16 data rows (slab rows 1..16); the
                # constant ws=0/17 pad rows were memset once up front.
                st = stps[cm2].tile([128, 16, 130], BF16, name="st", tag="st")
                nc.vector.memset(st[:, :, 0], 0.0)
                nc.vector.memset(st[:, :, 129], 0.0)
                for ybg in range(4):
                    dst_sl = st[:, 4 * ybg : 4 * ybg + 4, 1:129]
                    if ybg % 2 == 0:
                        nc.vector.tensor_copy(dst_sl, pts[ybg][:])
                    else:
                        nc.scalar.copy(dst_sl, pts[ybg][:])
                # st[(8ct+j), ws, x] -> corr slabs (c, j), split by ct-half.
                # Shuffles ride gpsimd SWDGE rings (DMA engines 4-15, which
                # sit idle) so the 4 engines behind the HWDGE queues carry
                # only bt loads; the last cms use the by-then-idle HWDGE
                # queues so the SWDGE path is never the stage-A tail.
                c, cmh = cm // 8, cm % 8
                if cm >= 14:
                    # quarters across all queues so the tail shuffles land
                    # fast (loads are done; HWDGE engines are idle by now)
                    for qi, eng in enumerate(
                        (nc.sync, nc.scalar, nc.gpsimd, nc.gpsimd)
                    ):
                        eng.dma_start(
                            out=corr_j[
                                16 * cmh + 4 * qi : 16 * cmh + 4 * qi + 4,
                                c * 8 : c * 8 + 8,
                                130 : 130 + 16 * 130,
                            ],
                            in_=st[32 * qi : 32 * qi + 32],
                        )
                else:
                    # both halves on gpsimd SWDGE (DMA engines 4-15): the
                    # HWDGE queues' 4 engines carry only bt loads
                    e0, e1 = nc.gpsimd, nc.gpsimd
                    e0.dma_start(
                        out=corr_j[
                            16 * cmh : 16 * cmh + 8,
                            c * 8 : c * 8 + 8,
                            130 : 130 + 16 * 130,
                        ],
                        in_=st[0:64],
                    )
                    e1.dma_start(
                        out=corr_j[
                            16 * cmh + 8 : 16 * cmh + 16,
                            c * 8 : c * 8 + 8,
                            130 : 130 + 16 * 130,
                        ],
                        in_=st[64:128],
                    )
                if pr >= 4:
                    # fill this cm's trailing DMA-stall window with stage-B
                    # chunk-0 work (its corr half has long been shuffled)
                    bk = 3 * (2 * (pr - 4) + cm2)
                    for d in range(3):
                        emit_c0_partial(bk + d)

    # ---- Stage B pass 2: chunk-1 taps; partialed blocks add their parked
    # chunk-0 sums in the combine op.  Stage A's PSUM banks are free now, so
    # a 4-deep pool hides the combine/out-DMA latency. ----
    psB2 = ctx.enter_context(tc.tile_pool(name="psB2", bufs=4, space="PSUM"))
    for k, (wg, j0p) in enumerate(blocks):
        w0, rw = W0S[wg], RWS[wg]
        n = rw * 130
        pb = psB2.tile([128, n], F32, name="pb2", tag="pb2")
        cs = (1,) if k < NPART else (0, 1)
        for c in cs:
            for dd in range(4):
                off0 = slab_off(c, 2 * j0p + dd - 1, w0)
                for kx in range(3):
                    nc.tensor.matmul(
                        pb[:],
                        w_sb[:, c * 12 + dd * 3 + kx, :, :],
                        corr_flat[:, off0 + kx : off0 + kx + n],
                        start=(c == cs[0] and dd == 0 and kx == 0),
                        stop=(c == 1 and dd == 3 and kx == 2),
                    )
        ot = outp.tile([128, rw, WM], F32, name="ot", tag="ot")
        pbv = pb[:].rearrange("p (a b) -> p a b", b=130)[:, :, 0:128]
        if k < NPART:
            # out = (pb + bias) + parked chunk-0 partial, one DVE op
            nc.vector.scalar_tensor_tensor(
                ot[:],
                pbv,
                b_sb[:, 0:1],
                bpx[:, k, :n].rearrange("p (a b) -> p a b", b=130)[:, :, 0:128],
                mybir.AluOpType.add,
                mybir.AluOpType.add,
            )
        else:
            nc.scalar.activation(
                ot[:],
                pbv,
                mybir.ActivationFunctionType.Identity,
                bias=b_sb[:, 0:1],
            )
        for sl in range(2):
            eng = nc.sync if sl == 0 else nc.scalar
            eng.dma_start(
                out=out_v[sl, :, j0p, w0 : w0 + rw, :],
                in_=ot[64 * sl : 64 * sl + 64],
            )


# revision 49
# speedup vs baseline: 1.1329x; 1.0968x over previous
"""DenseCorr2d full kernel for 8 Trainium2 NeuronCores.

Reference computation (per example b):
  corr[(cm*16+ct), y, x] = sum_{dy,dx} tm_edgepad[cm, y+dy, x+dx] * tp[ct, dy, dx]
  out[co, y, x] = bias[co] + sum_{ci,ky,kx} W[co, ci, ky, kx] * corr_zpad[ci, y+ky-1, x+kx-1]

Sharding: data-parallel over batch; core i computes example i entirely.

Stage A (dense correlation) folds the template taps into the matmul
contraction dim: with y = 8w + j and dx = 4a + de, the contraction rows are
(f = j+dy, de) = 92 partitions, the stationary columns are (ct, j) = 128
(fully dense), and accumulation over a happens in PSUM (4 matmuls per
128-col x 512-spatial tile).  The moving operand baseT[cm, (f,de), w, x'] =
tm_pad[cm, 8w+f, x'+de] is precomputed on host so each tile load is a
contiguous-per-partition DMA.  bt loads ride the two HWDGE queues (sync:
cm2=0, scalar: cm2=1) as maximal-size DMAs, all issued up front;
PSUM is allocated per-cm so evacuation of cm N overlaps the matmuls of
cm N+1.

corr lives in SBUF j-interleaved: corr[ci, (c, j, ws, x)] holds row
y = 8*(ws-1) + j of chunk c (ws=0/17 are constant zero rows, memset once
up front; x is 130 wide with zero x-borders).  The PSUM evacuation is a
full-rate copy (cast fp32->bf16 into a staging tile) plus two half-slab
DMAs per cm on the gpsimd SWDGE rings (DMA engines 4-15), keeping the 4
engines behind the HWDGE queues free for bt loads.  The kernel is DMA-
bound in this phase, so the chunk-0 halves of ALL stage-B blocks are
interleaved into pairs 4-7 (3 per cm): they fill the PE's DMA-stall
windows, their bf16 partials parking in SBUF.  Pass 2 then runs only the
chunk-1 taps and folds partial+bias into the psum evacuation with one
DVE scalar_tensor_tensor per block.

Stage B runs the 3x3 'same' merge conv over residue bands as ROW-PAIRED
matmuls: one [128, 2, 64] stationary holds the weights for two adjacent
output rows (j0, j0+1) that share the same corr slab row jj = j0+dd-1
(slot s covers row j0+s with ky = dd-s; invalid (dd,s) combos hold zero
weights).  That cuts the matmuls per block from 18 to 12 per output row
and uses the full 128 PE columns on the interior taps.  Each slot
evacuates to its own partition range of the output tile (scalar adds the
bias), and the output DMA un-interleaves the (slot, co) partitions into
the right y rows.

All matmuls run in bf16 (inputs are unit-normal; accumulation in fp32
PSUM keeps the relative error ~4e-3, well inside the 2e-2 gate).
"""

from contextlib import ExitStack

import ml_dtypes
import numpy as np

import concourse.bass as bass
import concourse.tile as tile
from concourse import bacc, mybir
from concourse.bass_utils import run_bass_kernel_spmd

F32 = mybir.dt.float32
BF16 = mybir.dt.bfloat16

N_CORES = 8
# Problem shapes (hardcoded per contract).
B, CT, HT, WT = 8, 16, 16, 16
CM, HM, WM = 16, 128, 128
COUT, K = 64, 3
HP = HM + HT - 1  # 143 padded image rows/cols
NF = 23  # f = j + dy range
NP = 4 * NF  # 92 contraction rows (f, de); de in [0,4), dx = 4a + de
NA = 4  # PSUM accumulation steps over a
XW = 140  # x' range of baseT (x + 4a <= 127+12)
SLAB = 18 * 130  # corr slab (c, j): 18 ws-rows of 130
NCORR = 16 * SLAB  # 2 chunks * 8 j

# stage-B w-window split: 16 w values per j0-residue, psum <= 512 fp32
RWS = [3, 3, 3, 3, 2, 2]
W0S = [0, 3, 6, 9, 12, 14]

_CACHE: dict = {}


def _emit(ctx: ExitStack, tc, nc, btT, sa2, wst, bia, zzb, out):
    const = ctx.enter_context(tc.tile_pool(name="const", bufs=1))
    corrp = ctx.enter_context(tc.tile_pool(name="corrp", bufs=1))

    # sa2 rides the scalar queue so the sync queue's first op is the first
    # bt load; both are needed by matmul #1.  w goes to the otherwise-idle
    # gpsimd SWDGE so it never delays loads or shuffles.
    sa2_sb = const.tile([NP, NA, 128], BF16, name="sa2_sb")
    nc.scalar.dma_start(out=sa2_sb[:], in_=sa2.ap())
    b_sb = const.tile([128, 1], F32, name="b_sb")
    nc.scalar.dma_start(out=b_sb[:], in_=bia.ap())
    # row-paired stage-B weights: [k, (c, dd, kx), s, co]
    w_sb = const.tile([128, 24, 2, COUT], BF16, name="w_sb")
    nc.gpsimd.dma_start(out=w_sb[:, :12], in_=wst.ap()[:, :12])
    nc.gpsimd.dma_start(out=w_sb[:, 12:], in_=wst.ap()[:, 12:])

    corr_sb = corrp.tile([128, NCORR + 2], BF16, name="corr_sb")
    corr_flat = corr_sb[:]
    # slab view: [p, c*8+j, ws*130+x]
    corr_j = corr_sb[:, :NCORR].rearrange("p (s t) -> p s t", s=16)
    corr_r = corr_sb[:, :NCORR].rearrange("p (s w x) -> p s w x", s=16, w=18)
    # the ws=0/17 zero-pad rows are constant: memset once up front instead of
    # shuffling 2 zero rows per cm (-11% shuffle bytes)
    nc.gpsimd.memset(corr_r[:, :, 0, :], 0.0)
    nc.gpsimd.memset(corr_r[:, :, 17, :], 0.0)
    # tail slack
    nc.scalar.dma_start(out=corr_sb[:, NCORR:], in_=zzb.ap()[:, :2])

    # ---- Stage B structure (needed during stage A for the interleave) ----
    def slab_off(c, jj, w0):
        if jj < 0:
            return (c * 8 + 7) * SLAB + w0 * 130
        if jj > 7:
            return (c * 8 + jj - 8) * SLAB + (w0 + 2) * 130
        return (c * 8 + jj) * SLAB + (w0 + 1) * 130

    # out dram view for the un-interleave: [s, co, j0p, w, x]
    out_v = out.ap().rearrange("co (w jp s) x -> s co jp w x", jp=4, s=2)
    blocks = [(wg, j0p) for wg in range(len(RWS)) for j0p in range(4)]
    NPART = 24  # blocks whose chunk-0 half runs inside stage A
    psB = ctx.enter_context(tc.tile_pool(name="psB", bufs=2, space="PSUM"))
    outp = ctx.enter_context(tc.tile_pool(name="outp", bufs=3))
    bpp = ctx.enter_context(tc.tile_pool(name="bpp", bufs=1))
    # bf16 partials of the interleaved blocks (all rw=3 -> n=390); the
    # half-precision parking adds ~0.3% RMS, far inside the 2e-2 gate
    bpx = bpp.tile([128, NPART, 390], BF16, name="bpx")

    def emit_c0_partial(k):
        # chunk-0 half of block k, run inside stage A's DMA-stall windows;
        # the bf16 partial parks in SBUF until pass 2.
        wg, j0p = blocks[k]
        n = RWS[wg] * 130
        pb = psB.tile([128, n], F32, name="pb", tag="pb")
        for dd in range(4):
            off0 = slab_off(0, 2 * j0p + dd - 1, W0S[wg])
            for kx in range(3):
                nc.tensor.matmul(
                    pb[:],
                    w_sb[:, dd * 3 + kx, :, :],
                    corr_flat[:, off0 + kx : off0 + kx + n],
                    start=(dd == 0 and kx == 0),
                    stop=(dd == 3 and kx == 2),
                )
        if k % 2 == 0:
            nc.scalar.copy(bpx[:, k, :n], pb[:])
        else:
            nc.vector.tensor_copy(bpx[:, k, :n], pb[:])

    # ---- Stage A ----
    with (
        tc.tile_pool(name="btp", bufs=8) as btp,
        tc.tile_pool(name="stp0", bufs=4) as stp0,
        tc.tile_pool(name="stp1", bufs=4) as stp1,
        tc.tile_pool(name="psA", bufs=6, space="PSUM") as psA,
    ):
        stps = [stp0, stp1]

        def load_pair(pr):
            # big per-cm2-half DMAs, cm2=0 on sync / cm2=1 on scalar: the
            # two HWDGE queues pull in parallel (each sustains ~30GB/s;
            # one queue alone cannot feed the PE).
            t = btp.tile([NP, 2, 16, XW], BF16, name="bt", tag="bt")
            if pr == 0:
                nc.sync.dma_start(out=t[:, 0, 0:8], in_=btT.ap()[0, :, 0, 0:8])
                nc.sync.dma_start(out=t[:, 0, 8:16], in_=btT.ap()[0, :, 0, 8:16])
            else:
                nc.sync.dma_start(out=t[:, 0], in_=btT.ap()[pr, :, 0])
            nc.scalar.dma_start(out=t[:, 1], in_=btT.ap()[pr, :, 1])
            return t

        loads = {pr: load_pair(pr) for pr in range(8)}
        for pr in range(8):
            bt = loads.pop(pr)
            for cm2 in range(2):
                cm = 2 * pr + cm2
                pts = [
                    psA.tile([128, 4, 128], F32, name=f"pA{ybg}", tag="pA")
                    for ybg in range(4)
                ]
                # a-inner so each ybg-block only needs its own bt chunk; the
                # 4 accumulating matmuls to one bank run back-to-back.
                for ybg in range(4):
                    for a in range(NA):
                        nc.tensor.matmul(
                            pts[ybg][:],
                            sa2_sb[:, a, :],
                            bt[:, cm2, 4 * ybg : 4 * ybg + 4, 4 * a : 4 * a + 128],
                            start=(a == 0),
                            stop=(a == NA - 1),
                        )
                # st carries only the 16 data rows (slab rows 1..16); the
                # constant ws=0/17 pad rows were memset once up front.
                st = stps[cm2].tile([128, 16, 130], BF16, name="st", tag="st")
                nc.vector.memset(st[:, :, 0], 0.0)
                nc.vector.memset(st[:, :, 129], 0.0)
                for ybg in range(4):
                    dst_sl = st[:, 4 * ybg : 4 * ybg + 4, 1:129]
                    if ybg % 2 == 0:
                        nc.vector.tensor_copy(dst_sl, pts[ybg][:])
                    else:
                        nc.scalar.copy(dst_sl, pts[ybg][:])
                # st[(8ct+j), ws, x] -> corr slabs (c, j), split by ct-half.
                # Shuffles ride gpsimd SWDGE rings (DMA engines 4-15, which
                # sit idle) so the 4 engines behind the HWDGE queues carry
                # only bt loads; the last cms use the by-then-idle HWDGE
                # queues so the SWDGE path is never the stage-A tail.
                c, cmh = cm // 8, cm % 8
                if cm >= 14:
                    # quarters across all queues so the tail shuffles land
                    # fast (loads are done; HWDGE engines are idle by now)
                    for qi, eng in enumerate(
                        (nc.sync, nc.scalar, nc.gpsimd, nc.gpsimd)
                    ):
                        eng.dma_start(
                            out=corr_j[
                                16 * cmh + 4 * qi : 16 * cmh + 4 * qi + 4,
                                c * 8 : c * 8 + 8,
                                130 : 130 + 16 * 130,
                            ],
                            in_=st[32 * qi : 32 * qi + 32],
                        )
                else:
                    # both halves on gpsimd SWDGE (DMA engines 4-15): the
                    # HWDGE queues' 4 engines carry only bt loads
                    e0, e1 = nc.gpsimd, nc.gpsimd
                    e0.dma_start(
                        out=corr_j[
                            16 * cmh : 16 * cmh + 8,
                            c * 8 : c * 8 + 8,
                            130 : 130 + 16 * 130,
                        ],
                        in_=st[0:64],
                    )
                    e1.dma_start(
                        out=corr_j[
                            16 * cmh + 8 : 16 * cmh + 16,
                            c * 8 : c * 8 + 8,
                            130 : 130 + 16 * 130,
                        ],
                        in_=st[64:128],
                    )
                if pr >= 4:
                    # fill this cm's trailing DMA-stall window with stage-B
                    # chunk-0 work (its corr half has long been shuffled)
                    bk = 3 * (2 * (pr - 4) + cm2)
                    for d in range(3):
                        emit_c0_partial(bk + d)

    # ---- Stage B pass 2: chunk-1 taps; partialed blocks add their parked
    # chunk-0 sums in the combine op.  Stage A's PSUM banks are free now, so
    # a 4-deep pool hides the combine/out-DMA latency. ----
    psB2 = ctx.enter_context(tc.tile_pool(name="psB2", bufs=4, space="PSUM"))
    for k, (wg, j0p) in enumerate(blocks):
        w0, rw = W0S[wg], RWS[wg]
        n = rw * 130
        pb = psB2.tile([128, n], F32, name="pb2", tag="pb2")
        cs = (1,) if k < NPART else (0, 1)
        for c in cs:
            for dd in range(4):
                off0 = slab_off(c, 2 * j0p + dd - 1, w0)
                for kx in range(3):
                    nc.tensor.matmul(
                        pb[:],
                        w_sb[:, c * 12 + dd * 3 + kx, :, :],
                        corr_flat[:, off0 + kx : off0 + kx + n],
                        start=(c == cs[0] and dd == 0 and kx == 0),
                        stop=(c == 1 and dd == 3 and kx == 2),
                    )
        ot = outp.tile([128, rw, WM], F32, name="ot", tag="ot")
        pbv = pb[:].rearrange("p (a b) -> p a b", b=130)[:, :, 0:128]
        if k < NPART:
            # out = (pb + bias) + parked chunk-0 partial, one DVE op
            nc.vector.scalar_tensor_tensor(
                ot[:],
                pbv,
                b_sb[:, 0:1],
                bpx[:, k, :n].rearrange("p (a b) -> p a b", b=130)[:, :, 0:128],
                mybir.AluOpType.add,
                mybir.AluOpType.add,
            )
        else:
            nc.scalar.activation(
                ot[:],
                pbv,
                mybir.ActivationFunctionType.Identity,
                bias=b_sb[:, 0:1],
            )
        for sl in range(2):
            eng = nc.sync if sl == 0 else nc.scalar
            eng.dma_start(
                out=out_v[sl, :, j0p, w0 : w0 + rw, :],
                in_=ot[64 * sl : 64 * sl + 64],
            )
